# revision 1
# baseline (speedup 1.0000x reference)
"""GCNConv kernel for Trainium2, 8 NeuronCores, graph/data-parallel by destination node.

Math (matches the PyG GCNConv reference):
    drop pre-existing self loops; deg[i] = #non-self edges with row==i, +1
    dinv = deg**-0.5
    out[d] = dinv[d] * ( sum_{e: row[e]==d} dinv[col[e]]*x[col[e]]  +  dinv[d]*x[d] ) @ W + bias

Strategy:
  * Host: compute deg/dinv (O(E) bincount), pre-scale x' = dinv*x, partition
    destination nodes across 8 cores, bin-pack each core's 12500 dests into
    196 blocks of <=64 (balanced edge counts), sort edges by
    (chunk, source-bank, dest) and emit int16 gather index streams
    (bank-local, since the HW gather instruction takes int16 indices).
  * Device (per core, identical program - SPMD):
      - dma_gather x'[col] rows (512B each) from HBM per (chunk, bank)
      - build one-hot [128 edges x 64 dests] tiles on DVE (is_equal vs iota)
      - PE matmul-accumulate:  psum[feat, dest64] += V_tile^T-contract-onehot
      - self-loop term added via identity matmul of the (permuted) x' rows
      - apply W (PE), dest-side dinv scale + bias (DVE), DMA out
  * Host: un-permute rows of the per-core outputs into the full [100000,128].
"""

import sys

for _p in ("/opt/trn_rl_repo", "/root/.axon_site/_ro/trn_rl_repo"):
    if _p not in sys.path:
        sys.path.append(_p)

import heapq
import os

import numpy as np

N_NODES = 100000
N_EDGES = 1600000
D = 128
NC = 8
BLK = 64          # dests per one-hot window / psum tile
BPC = 8           # 64-blocks per chunk
BANK = 32768      # gather bank size (int16 index reach)
CALL_TILES = 8    # tiles (of 128 idx) per dma_gather call (ucode cap 1024 idx)
NQ = int(os.environ.get("GCN_NQ", "3"))  # SWDGE queues for gather calls


def _prep(x, edge_index):
    """Host-side preprocessing. Returns (cfg, per_core, shared) where cfg has
    the compile-time structure (uniform across cores) and per_core the data."""
    N = x.shape[0]
    PART = N // NC
    NBLK = -(-PART // BLK)          # 64-blocks per core
    NCH = -(-NBLK // BPC)           # chunks per core
    NDEST = NBLK * BLK              # padded dest slots per core
    NBANK = -(-N // BANK)
    CHD = BPC * BLK                 # dests per chunk (512)

    row = np.asarray(edge_index[0]).astype(np.int64)
    col = np.asarray(edge_index[1]).astype(np.int64)
    ns = row != col
    er = row[ns]
    ec = col[ns]
    deg = np.bincount(er, minlength=N).astype(np.float32) + 1.0
    dinv = deg ** -0.5
    xprime = np.asarray(x, dtype=np.float32) * dinv[:, None]

    core = er // PART
    per_core_raw = []
    for m in range(NC):
        sel = core == m
        dl = er[sel] - m * PART
        c_ = ec[sel]
        dcnt = np.bincount(dl, minlength=PART)
        # balanced bin packing of dests into NBLK bins of <= BLK slots
        order = np.argsort(-dcnt, kind="stable")
        heap = [(0, b) for b in range(NBLK)]
        heapq.heapify(heap)
        fill = np.zeros(NBLK, np.int64)
        newid = np.empty(PART, np.int64)
        for d in order:
            tot, b = heapq.heappop(heap)
            newid[d] = b * BLK + fill[b]
            fill[b] += 1
            if fill[b] < BLK:
                heapq.heappush(heap, (tot + int(dcnt[d]), b))
        dest_of = np.full(NDEST, -1, np.int64)
        dest_of[newid] = np.arange(PART)

        dn = newid[dl]
        bank = c_ >> 15
        ch = dn // CHD
        o = np.lexsort((dn, bank, ch))
        dn_s = dn[o]
        key_s = ch[o] * NBANK + bank[o]
        idxloc = (c_[o] & (BANK - 1)).astype(np.int16)
        cnt = np.bincount(key_s, minlength=NCH * NBANK).reshape(NCH, NBANK)
        per_core_raw.append(dict(dest_of=dest_of, dn_s=dn_s, key_s=key_s,
                                 idxloc=idxloc, cnt=cnt))

    cnt_max = np.max([pc["cnt"] for pc in per_core_raw], axis=0)
    ntiles = -(-cnt_max // 128)            # [NCH, NBANK] tiles per stream
    caps = ntiles * 128
    soff = np.zeros((NCH, NBANK), np.int64)
    flat = caps.ravel()
    soff.ravel()[1:] = np.cumsum(flat)[:-1]
    NSLOT = int(flat.sum())

    # per-(chunk, block64, bank) tile ranges, unioned over cores
    ranges = [[dict() for _ in range(BPC)] for _ in range(NCH)]
    per_core = []
    for m in range(NC):
        pc = per_core_raw[m]
        dn_s, key_s, idxloc = pc["dn_s"], pc["key_s"], pc["idxloc"]
        cnt = pc["cnt"]
        starts = np.zeros(NCH * NBANK, np.int64)
        starts[1:] = np.cumsum(cnt.ravel())[:-1]
        rank = np.arange(len(dn_s)) - starts[key_s]
        slots = soff.ravel()[key_s] + rank
        idx_flat = np.zeros(NSLOT, np.int16)
        idx_flat[slots] = idxloc
        destv_flat = np.full(NSLOT, -1.0, np.float32)
        destv_flat[slots] = (dn_s - (dn_s // CHD) * CHD).astype(np.float32)

        for c in range(NCH):
            for k in range(NBANK):
                n = cnt[c, k]
                if n == 0:
                    continue
                g0 = starts[c * NBANK + k]
                seg = dn_s[g0:g0 + n]
                nb64 = min(BPC, NBLK - c * BPC)
                bnds = np.searchsorted(seg, c * CHD + np.arange(nb64 + 1) * BLK)
                for bb in range(nb64):
                    p0, p1 = bnds[bb], bnds[bb + 1]
                    if p0 == p1:
                        continue
                    t0, t1 = p0 // 128, -(-p1 // 128)
                    cur = ranges[c][bb].get(k)
                    if cur is None:
                        ranges[c][bb][k] = [t0, t1]
                    else:
                        cur[0] = min(cur[0], t0)
                        cur[1] = max(cur[1], t1)

        idx16 = np.tile(idx_flat.reshape(-1, 16).T, (NC, 1))        # [128, NSLOT//16]
        destv = destv_flat.reshape(-1, 128).T.copy()                # [128, NSLOT//128]
        dest_of = pc["dest_of"]
        valid = dest_of >= 0
        gid = np.where(valid, m * PART + dest_of, 0)
        xpp = np.where(valid[:, None], xprime[gid], 0.0).astype(np.float32)
        NB128 = NBLK // 2
        dvb = np.where(valid, dinv[gid], 0.0).astype(np.float32)
        dinvb = dvb.reshape(NB128, 128).T.copy()                    # [128, NB128]
        per_core.append(dict(idx16=idx16, destv=destv, xpp=xpp, dinvb=dinvb,
                             dest_of=dest_of))

    Rlist = [[sorted((k, v[0], v[1]) for k, v in ranges[c][bb].items())
              for bb in range(BPC)] for c in range(NCH)]
    cfg = dict(N=N, PART=PART, NBLK=NBLK, NCH=NCH, NDEST=NDEST, NBANK=NBANK,
               NSLOT=NSLOT, ntiles=ntiles, soff=soff, R=Rlist)
    n_inst = sum(t1 - t0 for c in range(NCH) for bb in range(BPC)
                 for (_, t0, t1) in Rlist[c][bb])
    cfg["n_inst"] = n_inst
    shared = dict(xprime=xprime)
    return cfg, per_core, shared


def _build(cfg):
    from concourse import bacc, tile
    import concourse.mybir as mybir

    N = cfg["N"]
    NCH, NBANK, NSLOT, NDEST = cfg["NCH"], cfg["NBANK"], cfg["NSLOT"], cfg["NDEST"]
    NBLK = cfg["NBLK"]
    NB128 = NBLK // 2
    ntiles, soff, R = cfg["ntiles"], cfg["soff"], cfg["R"]
    f32 = mybir.dt.float32
    CHD = BPC * BLK

    nc = bacc.Bacc("TRN2", target_bir_lowering=False, debug=False, num_devices=NC,
                   num_swdge_queues=NQ)
    banks = []
    for k in range(NBANK):
        rows = min(BANK, N - k * BANK)
        banks.append(nc.dram_tensor(f"xb{k}", [rows, D], f32,
                                    kind="ExternalInput").ap())
    xpp = nc.dram_tensor("xpp", [NDEST, D], f32, kind="ExternalInput").ap()
    idx = nc.dram_tensor("idx", [128, NSLOT // 16], mybir.dt.int16,
                         kind="ExternalInput").ap()
    dv = nc.dram_tensor("dv", [128, NSLOT // 128], f32, kind="ExternalInput").ap()
    iota = nc.dram_tensor("iota", [128, CHD], f32, kind="ExternalInput").ap()
    identd = nc.dram_tensor("identd", [128, BLK], f32, kind="ExternalInput").ap()
    wmat = nc.dram_tensor("wmat", [D, D], f32, kind="ExternalInput").ap()
    biasb = nc.dram_tensor("biasb", [128, D], f32, kind="ExternalInput").ap()
    dinvb = nc.dram_tensor("dinvb", [128, NB128], f32, kind="ExternalInput").ap()
    outp = nc.dram_tensor("outp", [NDEST, D], f32, kind="ExternalOutput").ap()

    self_qn = [0]
    with tile.TileContext(nc) as tc:
        with tc.tile_pool(name="const", bufs=1) as cp, \
             tc.tile_pool(name="stage", bufs=16) as sp, \
             tc.tile_pool(name="oh", bufs=8) as ohp, \
             tc.tile_pool(name="psA", bufs=4, space="PSUM") as pa, \
             tc.tile_pool(name="psB", bufs=2, space="PSUM") as pb, \
             tc.tile_pool(name="work", bufs=3) as wp:
            iota_sb = cp.tile([128, CHD], f32)
            nc.sync.dma_start(out=iota_sb[:], in_=iota[:])
            identd_sb = cp.tile([128, BLK], f32)
            nc.sync.dma_start(out=identd_sb[:], in_=identd[:])
            w_sb = cp.tile([D, D], f32)
            nc.sync.dma_start(out=w_sb[:], in_=wmat[:])
            biasb_sb = cp.tile([128, D], f32)
            nc.sync.dma_start(out=biasb_sb[:], in_=biasb[:])
            dinvb_sb = cp.tile([128, NB128], f32)
            nc.sync.dma_start(out=dinvb_sb[:], in_=dinvb[:])
            idx_sb = cp.tile([128, NSLOT // 16], mybir.dt.int16)
            nc.sync.dma_start(out=idx_sb[:], in_=idx[:])
            dv_sb = cp.tile([128, NSLOT // 128], f32)
            nc.sync.dma_start(out=dv_sb[:], in_=dv[:])

            for c in range(NCH):
                nb64 = min(BPC, NBLK - c * BPC)
                nb128 = nb64 // 2
                xp_t = wp.tile([128, nb128, D], f32, tag="xp")
                nc.sync.dma_start(
                    out=xp_t[:],
                    in_=xpp[c * CHD: c * CHD + nb64 * BLK].rearrange(
                        "(n p) d -> p n d", p=128))
                # stages[k] = (list of (call_tile, tiles_in_call), stream slot off)
                stages = {}
                for k in range(NBANK):
                    nt = int(ntiles[c][k])
                    if nt == 0:
                        continue
                    so = int(soff[c][k])
                    calls = []
                    for j in range(0, nt, CALL_TILES):
                        ct = min(CALL_TILES, nt - j)
                        st = sp.tile([128, ct, D], f32, tag="st")
                        cso = so + j * 128
                        nidx = ct * 128
                        nc.gpsimd.dma_gather(
                            st[:], banks[k],
                            idx_sb[:, cso // 16: cso // 16 + nidx // 16],
                            num_idxs=nidx, num_idxs_reg=nidx, elem_size=D,
                            queue_num=self_qn[0] % NQ)
                        self_qn[0] += 1
                        calls.append(st)
                    stages[k] = (calls, so)
                hT = None
                for bb in range(nb64):
                    ps = pa.tile([128, BLK], f32)
                    first = True
                    for (k, t0, t1) in R[c][bb]:
                        calls, so = stages[k]
                        for t in range(t0, t1):
                            oh = ohp.tile([128, BLK], f32)
                            dvc = so // 128 + t
                            nc.vector.tensor_tensor(
                                out=oh[:],
                                in0=dv_sb[:, dvc:dvc + 1].to_broadcast([128, BLK]),
                                in1=iota_sb[:, bb * BLK:(bb + 1) * BLK],
                                op=mybir.AluOpType.is_equal)
                            st = calls[t // CALL_TILES]
                            nc.tensor.matmul(out=ps[:],
                                             lhsT=st[:, t % CALL_TILES, :],
                                             rhs=oh[:], start=first, stop=False)
                            first = False
                    h = bb % 2
                    nb = bb // 2
                    nc.tensor.matmul(
                        out=ps[:],
                        lhsT=xp_t[BLK * h: BLK * (h + 1), nb, :],
                        rhs=identd_sb[BLK * h: BLK * (h + 1), :],
                        start=first, stop=True)
                    if h == 0:
                        hT = wp.tile([128, 128], f32, tag="hT")
                    nc.vector.tensor_copy(out=hT[:, BLK * h: BLK * (h + 1)], in_=ps[:])
                    if h == 1:
                        B = c * (BPC // 2) + nb
                        po = pb.tile([128, D], f32)
                        nc.tensor.matmul(out=po[:], lhsT=hT[:], rhs=w_sb[:],
                                         start=True, stop=True)
                        osb = wp.tile([128, D], f32, tag="osb")
                        nc.vector.tensor_scalar(
                            out=osb[:], in0=po[:],
                            scalar1=dinvb_sb[:, B:B + 1], scalar2=None,
                            op0=mybir.AluOpType.mult)
                        nc.vector.tensor_tensor(
                            out=osb[:], in0=osb[:], in1=biasb_sb[:],
                            op=mybir.AluOpType.add)
                        nc.scalar.dma_start(out=outp[B * 128:(B + 1) * 128, :],
                                            in_=osb[:])
    nc.compile()
    return nc


def _run(x, edge_index, weight, bias, trace=False):
    K_BANK = BANK
    from concourse import bass_utils

    cfg, per_core, shared = _prep(x, edge_index)
    nc = _build(cfg)
    CHD = BPC * BLK
    iota_np = np.tile(np.arange(CHD, dtype=np.float32), (128, 1))
    identd_np = np.zeros((128, BLK), np.float32)
    identd_np[np.arange(128), np.arange(128) % BLK] = 1.0
    biasb_np = np.tile(np.asarray(bias, np.float32)[None, :], (128, 1))
    w_np = np.asarray(weight, np.float32)
    in_maps = []
    for m in range(NC):
        pc = per_core[m]
        im = dict(
            xpp=pc["xpp"], idx=pc["idx16"], dv=pc["destv"], iota=iota_np,
            identd=identd_np, wmat=w_np, biasb=biasb_np, dinvb=pc["dinvb"])
        xp = shared["xprime"]
        for k in range((xp.shape[0] + K_BANK - 1) // K_BANK):
            im[f"xb{k}"] = np.ascontiguousarray(
                xp[k * K_BANK: min((k + 1) * K_BANK, xp.shape[0])])
        in_maps.append(im)
    res = bass_utils.run_bass_kernel_spmd(
        nc, in_maps, core_ids=list(range(NC)), trace=trace)
    N = cfg["N"]
    PART = cfg["PART"]
    out = np.empty((N, D), np.float32)
    for m in range(NC):
        dest_of = per_core[m]["dest_of"]
        valid = dest_of >= 0
        out[m * PART + dest_of[valid]] = res.results[m]["outp"][valid]
    return out, res, cfg


def kernel(x, edge_index, weight, bias):
    out, _, _ = _run(x, edge_index, weight, bias, trace=False)
    return out



# revision 3
# speedup vs baseline: 1.4207x; 1.4207x over previous
"""GCNConv kernel for Trainium2, 8 NeuronCores, graph/data-parallel by destination node.

Math (matches the PyG GCNConv reference):
    drop pre-existing self loops; deg[i] = #non-self edges with row==i, +1
    dinv = deg**-0.5
    out[d] = dinv[d] * ( sum_{e: row[e]==d} dinv[col[e]]*x[col[e]]  +  dinv[d]*x[d] ) @ W + bias

Strategy:
  * Host: compute deg/dinv (O(E) bincount), pre-scale x' = dinv*x (stored fp16),
    partition destination nodes across 8 cores, bin-pack each core's 12500 dests
    into 196 blocks of <=64 (balanced edge counts), group blocks into 7 chunks of
    28 (1792 dests), sort edges by (chunk, source-bank, dest) and emit int16
    gather index streams (bank-local, since the HW gather instruction takes
    int16 indices; 1024 idx per call is the ucode cap).
  * Device (per core, identical program - SPMD):
      - dma_gather x'[col] fp16 rows (256B each) from HBM per (chunk, bank),
        4 SWDGE queues round-robin
      - build one-hot [128 edges x 128 dests] fp16 tiles on DVE
        (tensor_scalar is_equal against an iota row, per-partition dest id)
      - PE fp16 matmul-accumulate:  psum[feat, dest128] += st^T-contract-onehot
      - self-loop term added via identity matmul of the (permuted) x' rows
      - psum -> fp16 SBUF copy on the (otherwise idle) Activation engine
      - apply W (PE fp16), fused dest-side dinv scale + bias add (DVE
        scalar_tensor_tensor), DMA out from the Activation HWDGE queue
  * Host: un-permute rows of the per-core outputs into the full [100000,128].
"""

import sys

for _p in ("/opt/trn_rl_repo", "/root/.axon_site/_ro/trn_rl_repo"):
    if _p not in sys.path:
        sys.path.append(_p)

import heapq
import os

import numpy as np

N_NODES = 100000
N_EDGES = 1600000
D = 128
NC = 8
BLK = 64          # dests per bin-packing block
SB = 128          # dests per one-hot window / psum tile (superblock)
BPC = 28          # 64-blocks per chunk (7 chunks x 28 blocks = 196 exactly)
BANK = 32768      # gather bank size (int16 index reach)
CALL_TILES = 8    # tiles (of 128 idx) per dma_gather call (ucode cap 1024 idx)
NQ = int(os.environ.get("GCN_NQ", "4"))  # SWDGE queues for gather calls


def _prep(x, edge_index):
    """Host-side preprocessing. Returns (cfg, per_core, shared) where cfg has
    the compile-time structure (uniform across cores) and per_core the data."""
    N = x.shape[0]
    PART = N // NC
    NBLK = -(-PART // BLK)          # 64-blocks per core
    NCH = -(-NBLK // BPC)           # chunks per core
    NDEST = NBLK * BLK              # padded dest slots per core
    NBANK = -(-N // BANK)
    CHD = BPC * BLK                 # dests per chunk (1792)
    NSB = CHD // SB                 # superblocks per chunk (14)

    row = np.asarray(edge_index[0]).astype(np.int64)
    col = np.asarray(edge_index[1]).astype(np.int64)
    ns = row != col
    er = row[ns]
    ec = col[ns]
    deg = np.bincount(er, minlength=N).astype(np.float32) + 1.0
    dinv = deg ** -0.5
    xprime = (np.asarray(x, dtype=np.float32) * dinv[:, None]).astype(np.float16)

    core = er // PART
    per_core_raw = []
    for m in range(NC):
        sel = core == m
        dl = er[sel] - m * PART
        c_ = ec[sel]
        dcnt = np.bincount(dl, minlength=PART)
        # balanced bin packing of dests into NBLK bins of <= BLK slots
        order = np.argsort(-dcnt, kind="stable")
        heap = [(0, b) for b in range(NBLK)]
        heapq.heapify(heap)
        fill = np.zeros(NBLK, np.int64)
        newid = np.empty(PART, np.int64)
        for d in order:
            tot, b = heapq.heappop(heap)
            newid[d] = b * BLK + fill[b]
            fill[b] += 1
            if fill[b] < BLK:
                heapq.heappush(heap, (tot + int(dcnt[d]), b))
        dest_of = np.full(NDEST, -1, np.int64)
        dest_of[newid] = np.arange(PART)

        dn = newid[dl]
        bank = c_ >> 15
        ch = dn // CHD
        o = np.lexsort((dn, bank, ch))
        dn_s = dn[o]
        key_s = ch[o] * NBANK + bank[o]
        idxloc = (c_[o] & (BANK - 1)).astype(np.int16)
        cnt = np.bincount(key_s, minlength=NCH * NBANK).reshape(NCH, NBANK)
        per_core_raw.append(dict(dest_of=dest_of, dn_s=dn_s, key_s=key_s,
                                 idxloc=idxloc, cnt=cnt))

    cnt_max = np.max([pc["cnt"] for pc in per_core_raw], axis=0)
    ntiles = -(-cnt_max // 128)            # [NCH, NBANK] tiles per stream
    caps = ntiles * 128
    soff = np.zeros((NCH, NBANK), np.int64)
    flat = caps.ravel()
    soff.ravel()[1:] = np.cumsum(flat)[:-1]
    NSLOT = int(flat.sum())

    # per-(chunk, superblock, bank) tile ranges, unioned over cores
    ranges = [[dict() for _ in range(NSB)] for _ in range(NCH)]
    per_core = []
    for m in range(NC):
        pc = per_core_raw[m]
        dn_s, key_s, idxloc = pc["dn_s"], pc["key_s"], pc["idxloc"]
        cnt = pc["cnt"]
        starts = np.zeros(NCH * NBANK, np.int64)
        starts[1:] = np.cumsum(cnt.ravel())[:-1]
        rank = np.arange(len(dn_s)) - starts[key_s]
        slots = soff.ravel()[key_s] + rank
        idx_flat = np.zeros(NSLOT, np.int16)
        idx_flat[slots] = idxloc
        destv_flat = np.full(NSLOT, -1.0, np.float32)
        destv_flat[slots] = (dn_s - (dn_s // CHD) * CHD).astype(np.float32)

        for c in range(NCH):
            for k in range(NBANK):
                n = cnt[c, k]
                if n == 0:
                    continue
                g0 = starts[c * NBANK + k]
                seg = dn_s[g0:g0 + n]
                bnds = np.searchsorted(seg, c * CHD + np.arange(NSB + 1) * SB)
                for sb in range(NSB):
                    p0, p1 = bnds[sb], bnds[sb + 1]
                    if p0 == p1:
                        continue
                    t0, t1 = p0 // 128, -(-p1 // 128)
                    cur = ranges[c][sb].get(k)
                    if cur is None:
                        ranges[c][sb][k] = [t0, t1]
                    else:
                        cur[0] = min(cur[0], t0)
                        cur[1] = max(cur[1], t1)

        idx16 = np.tile(idx_flat.reshape(-1, 16).T, (NC, 1))        # [128, NSLOT//16]
        destv = destv_flat.reshape(-1, 128).T.copy()                # [128, NSLOT//128]
        dest_of = pc["dest_of"]
        valid = dest_of >= 0
        gid = np.where(valid, m * PART + dest_of, 0)
        xpp = np.where(valid[:, None], xprime[gid], np.float16(0.0)).astype(np.float16)
        NB128 = NDEST // 128
        dvb = np.where(valid, dinv[gid], 0.0).astype(np.float32)
        dinvb = dvb.reshape(NB128, 128).T.copy()                    # [128, NB128]
        per_core.append(dict(idx16=idx16, destv=destv, xpp=xpp, dinvb=dinvb,
                             dest_of=dest_of))

    Rlist = [[sorted((k, v[0], v[1]) for k, v in ranges[c][sb].items())
              for sb in range(NSB)] for c in range(NCH)]
    cfg = dict(N=N, PART=PART, NBLK=NBLK, NCH=NCH, NDEST=NDEST, NBANK=NBANK,
               NSB=NSB, NSLOT=NSLOT, ntiles=ntiles, soff=soff, R=Rlist)
    n_inst = sum(t1 - t0 for c in range(NCH) for sb in range(NSB)
                 for (_, t0, t1) in Rlist[c][sb])
    cfg["n_inst"] = n_inst
    shared = dict(xprime=xprime)
    return cfg, per_core, shared


def _build(cfg):
    from concourse import bacc, tile
    import concourse.mybir as mybir

    N = cfg["N"]
    NCH, NBANK, NSLOT, NDEST = cfg["NCH"], cfg["NBANK"], cfg["NSLOT"], cfg["NDEST"]
    NSB = cfg["NSB"]
    NB128 = NDEST // 128
    ntiles, soff, R = cfg["ntiles"], cfg["soff"], cfg["R"]
    f32 = mybir.dt.float32
    f16 = mybir.dt.float16
    CHD = BPC * BLK

    nc = bacc.Bacc("TRN2", target_bir_lowering=False, debug=False, num_devices=NC,
                   num_swdge_queues=NQ)
    banks = []
    for k in range(NBANK):
        rows = min(BANK, N - k * BANK)
        banks.append(nc.dram_tensor(f"xb{k}", [rows, D], f16,
                                    kind="ExternalInput").ap())
    xpp = nc.dram_tensor("xpp", [NDEST, D], f16, kind="ExternalInput").ap()
    idx = nc.dram_tensor("idx", [128, NSLOT // 16], mybir.dt.int16,
                         kind="ExternalInput").ap()
    dv = nc.dram_tensor("dv", [128, NSLOT // 128], f32, kind="ExternalInput").ap()
    iota = nc.dram_tensor("iota", [128, CHD], f16, kind="ExternalInput").ap()
    identd = nc.dram_tensor("identd", [128, SB], f16, kind="ExternalInput").ap()
    wmat = nc.dram_tensor("wmat", [D, D], f16, kind="ExternalInput").ap()
    biasb = nc.dram_tensor("biasb", [128, D], f32, kind="ExternalInput").ap()
    dinvb = nc.dram_tensor("dinvb", [128, NB128], f32, kind="ExternalInput").ap()
    outp = nc.dram_tensor("outp", [NDEST, D], f32, kind="ExternalOutput").ap()

    self_qn = [0]
    with tile.TileContext(nc) as tc:
        with tc.tile_pool(name="const", bufs=1) as cp, \
             tc.tile_pool(name="stage", bufs=24) as sp, \
             tc.tile_pool(name="oh", bufs=8) as ohp, \
             tc.tile_pool(name="psA", bufs=4, space="PSUM") as pa, \
             tc.tile_pool(name="psB", bufs=2, space="PSUM") as pb, \
             tc.tile_pool(name="work", bufs=3) as wp:
            iota_sb = cp.tile([128, CHD], f16)
            nc.sync.dma_start(out=iota_sb[:], in_=iota[:])
            identd_sb = cp.tile([128, SB], f16)
            nc.sync.dma_start(out=identd_sb[:], in_=identd[:])
            w_sb = cp.tile([D, D], f16)
            nc.sync.dma_start(out=w_sb[:], in_=wmat[:])
            biasb_sb = cp.tile([128, D], f32)
            nc.sync.dma_start(out=biasb_sb[:], in_=biasb[:])
            dinvb_sb = cp.tile([128, NB128], f32)
            nc.sync.dma_start(out=dinvb_sb[:], in_=dinvb[:])
            idx_sb = cp.tile([128, NSLOT // 16], mybir.dt.int16)
            nc.sync.dma_start(out=idx_sb[:], in_=idx[:])
            dv_sb = cp.tile([128, NSLOT // 128], f32)
            nc.sync.dma_start(out=dv_sb[:], in_=dv[:])

            for c in range(NCH):
                xp_t = wp.tile([128, NSB, D], f16, tag="xp")
                nc.sync.dma_start(
                    out=xp_t[:],
                    in_=xpp[c * CHD: (c + 1) * CHD].rearrange(
                        "(n p) d -> p n d", p=128))
                # stages[k] = (list of stage tiles (CALL_TILES each), slot off)
                # Issue calls round-robin across banks so stage tiles arrive
                # in roughly the order superblocks consume them (each
                # superblock reads the leading tiles of EVERY bank first) —
                # bank-major issue order deadlocks the stage pool.
                stages = {k: ([], int(soff[c][k]))
                          for k in range(NBANK) if int(ntiles[c][k]) != 0}
                maxcalls = max((-(-int(ntiles[c][k]) // CALL_TILES)
                                for k in stages), default=0)
                for j in range(maxcalls):
                    for k in stages:
                        nt = int(ntiles[c][k])
                        if j * CALL_TILES >= nt:
                            continue
                        ct = min(CALL_TILES, nt - j * CALL_TILES)
                        st = sp.tile([128, ct, D], f16, tag="st")
                        cso = int(soff[c][k]) + j * CALL_TILES * 128
                        nidx = ct * 128
                        nc.gpsimd.dma_gather(
                            st[:], banks[k],
                            idx_sb[:, cso // 16: cso // 16 + nidx // 16],
                            num_idxs=nidx, num_idxs_reg=nidx, elem_size=D,
                            queue_num=self_qn[0] % NQ)
                        self_qn[0] += 1
                        stages[k][0].append(st)
                for sb in range(NSB):
                    ps = pa.tile([128, SB], f32)
                    first = True
                    for (k, t0, t1) in R[c][sb]:
                        calls, so = stages[k]
                        for t in range(t0, t1):
                            oh = ohp.tile([128, SB], f16)
                            dvc = so // 128 + t
                            nc.vector.tensor_scalar(
                                out=oh[:],
                                in0=iota_sb[:, sb * SB:(sb + 1) * SB],
                                scalar1=dv_sb[:, dvc:dvc + 1], scalar2=None,
                                op0=mybir.AluOpType.is_equal)
                            st = calls[t // CALL_TILES]
                            nc.tensor.matmul(out=ps[:],
                                             lhsT=st[:, t % CALL_TILES, :],
                                             rhs=oh[:], start=first, stop=False)
                            first = False
                    # self-loop: += x'[dest]^T (identity matmul over the block)
                    nc.tensor.matmul(
                        out=ps[:], lhsT=xp_t[:, sb, :], rhs=identd_sb[:],
                        start=first, stop=True)
                    B = c * NSB + sb
                    hT = wp.tile([128, SB], f16, tag="hT")
                    nc.scalar.copy(out=hT[:], in_=ps[:])
                    po = pb.tile([128, D], f32)
                    nc.tensor.matmul(out=po[:], lhsT=hT[:], rhs=w_sb[:],
                                     start=True, stop=True)
                    osb = wp.tile([128, D], f32, tag="osb")
                    nc.vector.scalar_tensor_tensor(
                        out=osb[:], in0=po[:],
                        scalar=dinvb_sb[:, B:B + 1], in1=biasb_sb[:],
                        op0=mybir.AluOpType.mult, op1=mybir.AluOpType.add)
                    nc.scalar.dma_start(out=outp[B * 128:(B + 1) * 128, :],
                                        in_=osb[:])
    nc.compile()
    return nc


def _run(x, edge_index, weight, bias, trace=False):
    K_BANK = BANK
    from concourse import bass_utils

    cfg, per_core, shared = _prep(x, edge_index)
    nc = _build(cfg)
    CHD = BPC * BLK
    iota_np = np.tile(np.arange(CHD, dtype=np.float16), (128, 1))
    identd_np = np.zeros((128, SB), np.float16)
    identd_np[np.arange(128), np.arange(128) % SB] = np.float16(1.0)
    biasb_np = np.tile(np.asarray(bias, np.float32)[None, :], (128, 1))
    w_np = np.asarray(weight, np.float32).astype(np.float16)
    in_maps = []
    for m in range(NC):
        pc = per_core[m]
        im = dict(
            xpp=pc["xpp"], idx=pc["idx16"], dv=pc["destv"], iota=iota_np,
            identd=identd_np, wmat=w_np, biasb=biasb_np, dinvb=pc["dinvb"])
        xp = shared["xprime"]
        for k in range((xp.shape[0] + K_BANK - 1) // K_BANK):
            im[f"xb{k}"] = np.ascontiguousarray(
                xp[k * K_BANK: min((k + 1) * K_BANK, xp.shape[0])])
        in_maps.append(im)
    res = bass_utils.run_bass_kernel_spmd(
        nc, in_maps, core_ids=list(range(NC)), trace=trace)
    N = cfg["N"]
    PART = cfg["PART"]
    out = np.empty((N, D), np.float32)
    for m in range(NC):
        dest_of = per_core[m]["dest_of"]
        valid = dest_of >= 0
        out[m * PART + dest_of[valid]] = res.results[m]["outp"][valid]
    return out, res, cfg


def kernel(x, edge_index, weight, bias):
    out, _, _ = _run(x, edge_index, weight, bias, trace=False)
    return out
